# revision 1
# baseline (speedup 1.0000x reference)
"""BiMamba adapter Trainium2 kernel.

Sharding: 8 cores = (batch 2) x (direction 2) x (d_inner half 2).
Each core runs an identical SPMD program on its own weight slices:
  LN -> in_proj (u full + z half) -> causal conv + silu -> x_proj ->
  dt softplus -> selective scan (16 N-states, tensor_tensor_scan) ->
  gate -> fused out_proj+adapter_proj -> partial q (768, 2048).
Host reverses the backward-direction inputs/outputs and sums the 8
partials into the residual output.

d_inner channels are permuted host-side so each core's own half lives
in u-chunks 0..5 (keeps the device program identical across cores).

This deployment pays a large fixed cost per DVE instruction, so the
design batches the six 128-channel chunks into single wide (128, 12288)
tiles and runs one scan per state n; chunk boundaries are handled by
poisoning dt at each chunk's first column (dA -> 0 there, which resets
the recurrence state exactly since h starts at 0).
"""
import numpy as np

import concourse.bass as bass
import concourse.bacc as bacc
import concourse.tile as tile
from concourse import mybir
from concourse.bass_utils import run_bass_kernel_spmd

F16 = mybir.dt.float16
F32 = mybir.dt.float32
OP = mybir.AluOpType
AF = mybir.ActivationFunctionType

L = 2048
DM = 768          # d_model
DI = 1536         # d_inner
DH = 768          # d_inner half per core
DTR = 48          # dt rank
NS = 16           # d_state
NT = L // 128     # 16 token tiles
NDM = DM // 128   # 6
NDU = DI // 128   # 12
NDH = DH // 128   # 6
KC = 4            # conv taps
FC = 512          # psum free chunk
NFC = L // FC     # 4
WB = NDH * L      # 12288 batched free size
UP = L + 4        # padded upre block


def _build_program(rep=1, variant="full"):
    nc = bacc.Bacc("TRN2", target_bir_lowering=False, debug=False, num_devices=8)

    def din(name, shape, dt):
        return nc.dram_tensor(name, shape, dt, kind="ExternalInput").ap()

    aps = dict(
        xin=din("xin", [L, DM], F32),
        wuT=din("wuT", [DM, DI], F16),
        wzT=din("wzT", [DM, DH], F16),
        bu=din("bu", [128, NDU], F32),
        bz=din("bz", [128, NDH], F32),
        convw=din("convw", [128, NDU * KC], F32),
        convb=din("convb", [128, NDU], F32),
        xprojT=din("xprojT", [128, NDU * 80], F16),
        dtwT=din("dtwT", [DTR, DH], F16),
        dtb=din("dtb", [128, NDH], F32),
        dvec=din("dvec", [128, NDH], F32),
        w2T=din("w2T", [128, NDH * DM], F16),
        ident=din("ident", [128, 128], F16),
        qout=nc.dram_tensor("q", [DM, L], F32, kind="ExternalOutput").ap(),
    )
    # scratch DRAM to spill the silu(z) gate between phases
    aps["sgd"] = nc.dram_tensor("sgd", [128, WB], F16).ap()

    with tile.TileContext(nc) as tc:
        for _ in range(rep):
            _body(tc, nc, aps, variant)
    nc.compile()
    return nc


def _body(tc, nc, aps, variant="full"):
    xin, qout = aps["xin"], aps["qout"]

    with tc.tile_pool(name="params", bufs=1) as pp, \
         tc.tile_pool(name="big", bufs=1) as bigp, \
         tc.tile_pool(name="mmps", bufs=4, space=bass.MemorySpace.PSUM) as mmps:

        def load_param(name, src_ap, free, dt=F32):
            t = pp.tile([128, free], dt, tag=name)
            nc.sync.dma_start(t[:], src_ap)
            return t

        bu_sb = load_param("bu", aps["bu"], NDU)
        bz_sb = load_param("bz", aps["bz"], NDH)
        convb_sb = load_param("convb", aps["convb"], NDU)
        dtb_sb = load_param("dtb", aps["dtb"], NDH)
        dvec_sb = load_param("dvec", aps["dvec"], NDH)
        convw_sb = load_param("convw", aps["convw"], NDU * KC)
        id_sb = load_param("ident", aps["ident"], 128, F16)
        eps_sb = pp.tile([128, 1], F32, tag="eps")
        nc.vector.memset(eps_sb[:], 1e-5)

        dt_big = bigp.tile([128, WB], F16, tag="dt_big")
        v_big = bigp.tile([128, WB], F16, tag="v_big")
        yacc = bigp.tile([128, WB], F16, tag="yacc")
        xdT = bigp.tile([80, L], F16, tag="xdT")

        # ================= phase A: LN, in_proj, conv, xproj =================
        with tc.tile_pool(name="hT", bufs=1) as hTp, \
             tc.tile_pool(name="ubig", bufs=1) as ubigp:
            hT = [hTp.tile([128, L], F16, tag=f"hT{m}", name=f"hT{m}")
                  for m in range(NDM)]
            u_big = ubigp.tile([128, WB], F16, tag="u_big")
            sg_big = ubigp.tile([128, WB], F16, tag="sg_big")

            with tc.tile_pool(name="pa", bufs=3) as pa, \
                 tc.tile_pool(name="lns", bufs=4) as lns, \
                 tc.tile_pool(name="tpps", bufs=2,
                              space=bass.MemorySpace.PSUM) as tpps:
                rstd = lns.tile([128, NT], F32, tag="rstd", bufs=1)
                sdev = lns.tile([128, NT], F32, tag="sdev", bufs=1)
                xcs = []
                for tt in range(NT):
                    xt = pa.tile([128, DM], F32, tag="xt")
                    nc.sync.dma_start(xt[:], xin[tt * 128:(tt + 1) * 128, :])
                    scr = pa.tile([128, DM], F16, tag="scr")
                    musum = lns.tile([128, 1], F32, tag="musum")
                    nc.scalar.activation(scr[:], xt[:], AF.Copy,
                                         scale=-1.0 / DM, accum_out=musum[:])
                    xc = pa.tile([128, DM], F16, tag="xc", bufs=NT + 1,
                                 name="xc")
                    nc.scalar.activation(xc[:], xt[:], AF.Identity,
                                         bias=musum[:])
                    ssq = lns.tile([128, 1], F32, tag="ssq")
                    nc.scalar.activation(scr[:], xc[:], AF.Square,
                                         accum_out=ssq[:])
                    nc.scalar.activation(sdev[:, tt:tt + 1], ssq[:], AF.Sqrt,
                                         scale=1.0 / DM, bias=eps_sb[:])
                    xcs.append(xc)
                nc.vector.reciprocal(rstd[:], sdev[:])
                for tt in range(NT):
                    h16 = pa.tile([128, DM], F16, tag="h16")
                    nc.scalar.activation(h16[:], xcs[tt][:], AF.Copy,
                                         scale=rstd[:, tt:tt + 1])
                    for mc in range(NDM):
                        tp = tpps.tile([128, 128], F16, tag="tp")
                        nc.tensor.transpose(
                            tp[:], h16[:, mc * 128:(mc + 1) * 128], id_sb[:])
                        nc.scalar.activation(
                            hT[mc][:, tt * 128:(tt + 1) * 128], tp[:], AF.Copy)

            with tc.tile_pool(name="wch", bufs=1) as wchp, \
                 tc.tile_pool(name="upre", bufs=3) as uprep, \
                 tc.tile_pool(name="uoth", bufs=2) as uothp, \
                 tc.tile_pool(name="xdps", bufs=4,
                              space=bass.MemorySpace.PSUM) as xdps:
                wch = []
                for mc in range(NDM):
                    w = wchp.tile([128, DI + DH], F16, tag=f"wch{mc}",
                                  name=f"wch{mc}")
                    nc.sync.dma_start(w[:, :DI],
                                      aps["wuT"][mc * 128:(mc + 1) * 128, :])
                    nc.sync.dma_start(w[:, DI:],
                                      aps["wzT"][mc * 128:(mc + 1) * 128, :])
                    wch.append(w)
                xprojT_sb = wchp.tile([128, NDU * 80], F16, tag="xprojT")
                nc.sync.dma_start(xprojT_sb[:], aps["xprojT"])

                xdps_t = [xdps.tile([80, FC], F32, tag="xdps", name="xdps_t")
                          for _ in range(NFC)]
                for ic in range(NDU + NDH):
                    is_u = ic < NDU
                    if is_u:
                        ubuf = uprep.tile([128, UP], F16, tag="upre",
                                          name="ubuf")
                        nc.scalar.memzero(ubuf[:, 0:4])
                    for fc in range(NFC):
                        ps = mmps.tile([128, FC], F32, tag="mm", name="ps")
                        for mc in range(NDM):
                            nc.tensor.matmul(
                                ps[:], wch[mc][:, ic * 128:(ic + 1) * 128],
                                hT[mc][:, fc * FC:(fc + 1) * FC],
                                start=(mc == 0), stop=(mc == NDM - 1))
                        if is_u:
                            nc.scalar.activation(
                                ubuf[:, 4 + fc * FC:4 + (fc + 1) * FC],
                                ps[:], AF.Identity, bias=bu_sb[:, ic:ic + 1])
                        else:
                            zc = ic - NDU
                            nc.scalar.activation(
                                sg_big[:, zc * L + fc * FC:
                                       zc * L + (fc + 1) * FC],
                                ps[:], AF.Silu, bias=bz_sb[:, zc:zc + 1])
                    if is_u:
                        # conv: 4 taps, then silu in place; own half lives in
                        # u_big, other half is consumed by xproj immediately
                        if ic < NDH:
                            ub = u_big[:, ic * L:(ic + 1) * L]
                        else:
                            uo = uothp.tile([128, L], F16, tag="uoth",
                                            name="uoth")
                            ub = uo[:]
                        nc.vector.tensor_scalar_mul(
                            ub, ubuf[:, 1:1 + L],
                            convw_sb[:, ic * KC:ic * KC + 1])
                        for k in range(1, KC):
                            nc.vector.scalar_tensor_tensor(
                                ub, ubuf[:, 1 + k:1 + k + L],
                                convw_sb[:, ic * KC + k:ic * KC + k + 1], ub,
                                OP.mult, OP.add)
                        nc.scalar.activation(ub, ub, AF.Silu,
                                             bias=convb_sb[:, ic:ic + 1])
                        for fc in range(NFC):
                            nc.tensor.matmul(
                                xdps_t[fc][:],
                                xprojT_sb[:, ic * 80:(ic + 1) * 80],
                                ub[:, fc * FC:(fc + 1) * FC],
                                start=(ic == 0), stop=(ic == NDU - 1))
                for fc in range(NFC):
                    nc.scalar.activation(
                        xdT[:, fc * FC:(fc + 1) * FC], xdps_t[fc][:], AF.Copy)

            # dt = softplus(dtlow @ dtwT + dtb); yacc = u*D; v = dt*u
            with tc.tile_pool(name="dtw", bufs=1) as dtwp, \
                 tc.tile_pool(name="sp", bufs=3) as spp:
                dtw_sb = dtwp.tile([DTR, DH], F16, tag="dtw")
                nc.sync.dma_start(dtw_sb[:], aps["dtwT"])
                for mc in range(NDH):
                    for fc in range(NFC):
                        ps = mmps.tile([128, FC], F32, tag="mm", name="ps")
                        nc.tensor.matmul(
                            ps[:], dtw_sb[:, mc * 128:(mc + 1) * 128],
                            xdT[0:DTR, fc * FC:(fc + 1) * FC],
                            start=True, stop=True)
                        # softplus(x) = ln(1 + exp(x)); no Softplus ACT table
                        etmp = spp.tile([128, FC], F32, tag="etmp",
                                        name="etmp")
                        nc.scalar.activation(etmp[:], ps[:], AF.Exp,
                                             bias=dtb_sb[:, mc:mc + 1])
                        nc.scalar.activation(
                            dt_big[:, mc * L + fc * FC:mc * L + (fc + 1) * FC],
                            etmp[:], AF.Ln, bias=1.0)
                    nc.scalar.activation(
                        yacc[:, mc * L:(mc + 1) * L],
                        u_big[:, mc * L:(mc + 1) * L],
                        AF.Copy, scale=dvec_sb[:, mc:mc + 1])
                nc.vector.tensor_mul(v_big[:], dt_big[:], u_big[:])
            # spill gate to DRAM; u_big/sg_big freed with pool exit
            nc.sync.dma_start(aps["sgd"], sg_big[:])

        # poison dt at each chunk's first column: dA -> exp(-big) = 0 there,
        # which resets the batched scan state exactly at chunk boundaries.
        pois = dt_big[:].rearrange("p (c l) -> p c l", l=L)[:, :, 0:1]
        nc.vector.memset(pois, 60000.0)

        # ================= phase B: scan =================
        with tc.tile_pool(name="sw", bufs=1) as swp, \
             tc.tile_pool(name="bc", bufs=1) as bcp:

            def bcast_row(tag, row_ap):
                t = bcp.tile([128, WB], F16, tag=tag, name=tag)
                nc.sync.dma_start(t[0:1, 0:L], row_ap)
                for k in (1, 2, 4, 8, 16, 32, 64):
                    nc.sync.dma_start(t[k:2 * k, 0:L], t[0:k, 0:L])
                for j in range(1, NDH):
                    nc.sync.dma_start(t[:, j * L:(j + 1) * L], t[:, 0:L])
                return t

            for n in range(NS if variant != "noscan" else 0):
                bbc = bcast_row("bbc", xdT[DTR + n:DTR + n + 1, :])
                cbc = bcast_row("cbc", xdT[DTR + NS + n:DTR + NS + n + 1, :])
                at = swp.tile([128, WB], F16, tag="at", name="at")
                nc.scalar.activation(at[:], dt_big[:], AF.Exp,
                                     scale=-float(n + 1))
                nc.vector.tensor_mul(bbc[:], v_big[:], bbc[:])
                ht = swp.tile([128, WB], F16, tag="ht", name="ht")
                if variant == "nosc":
                    nc.vector.tensor_tensor(ht[:], at[:], bbc[:], OP.mult)
                else:
                    nc.vector.tensor_tensor_scan(
                        ht[:], at[:], bbc[:], 0.0, OP.mult, OP.add)
                nc.vector.tensor_mul(ht[:], ht[:], cbc[:])
                nc.vector.tensor_add(yacc[:], yacc[:], ht[:])

        # ================= phase C: gate + out_proj =================
        with tc.tile_pool(name="w2", bufs=1) as w2p, \
             tc.tile_pool(name="qs", bufs=3) as qsp:
            sgr = w2p.tile([128, WB], F16, tag="sgr")
            nc.sync.dma_start(sgr[:], aps["sgd"])
            nc.vector.tensor_mul(yacc[:], yacc[:], sgr[:])
            w2_sb = w2p.tile([128, NDH * DM], F16, tag="w2T")
            nc.sync.dma_start(w2_sb[:], aps["w2T"])
            for mc in range(NDM):
                for fc in range(NFC):
                    ps = mmps.tile([128, FC], F32, tag="mm", name="ps")
                    for kc in range(NDH):
                        nc.tensor.matmul(
                            ps[:],
                            w2_sb[:, kc * DM + mc * 128:
                                  kc * DM + (mc + 1) * 128],
                            yacc[:, kc * L + fc * FC:kc * L + (fc + 1) * FC],
                            start=(kc == 0), stop=(kc == NDH - 1))
                    qsb = qsp.tile([128, FC], F32, tag="qsb", name="qsb")
                    nc.scalar.activation(qsb[:], ps[:], AF.Copy)
                    nc.sync.dma_start(
                        qout[mc * 128:(mc + 1) * 128, fc * FC:(fc + 1) * FC],
                        qsb[:])


_CACHE = {}


def _get_program(rep=1, variant="full"):
    key = (rep, variant)
    if key not in _CACHE:
        _CACHE[key] = _build_program(rep, variant)
    return _CACHE[key]


def _prep_core_inputs(inp, b, d, half):
    f32 = np.float32
    f16 = np.float16
    pref = "mf" if d == 0 else "mb"
    g = lambda k: np.asarray(inp[f"{pref}_{k}"], f32)
    ln_w = np.asarray(inp["ln_w"], f32)
    ln_b = np.asarray(inp["ln_b"], f32)
    in_w = g("in_w")
    x = np.asarray(inp["x"], f32)[b]
    if d == 1:
        x = x[::-1]
    perm = np.concatenate([np.arange(half * DH, (half + 1) * DH),
                           np.arange((1 - half) * DH, (2 - half) * DH)])
    hs = slice(half * DH, (half + 1) * DH)
    wu = in_w[0:DI][perm]
    wz = in_w[DI + half * DH:DI + (half + 1) * DH]
    # device uses dA = exp(-(n+1)*dt); verify A really is -(n+1) per state
    A = -np.exp(g("A_log")[hs])                 # (DH, NS)
    assert np.abs(A + np.arange(1, NS + 1)).max() < 1e-4, \
        "kernel assumes A[:, n] == -(n+1)"
    return {
        "xin": np.ascontiguousarray(x, f32),
        "wuT": np.ascontiguousarray((wu.T * ln_w[:, None]).astype(f16)),
        "wzT": np.ascontiguousarray((wz.T * ln_w[:, None]).astype(f16)),
        "bu": np.ascontiguousarray((wu @ ln_b).reshape(NDU, 128).T, f32),
        "bz": np.ascontiguousarray((wz @ ln_b).reshape(NDH, 128).T, f32),
        "convw": np.ascontiguousarray(
            g("conv_w")[perm].reshape(NDU, 128, KC).transpose(1, 0, 2)
            .reshape(128, -1), f32),
        "convb": np.ascontiguousarray(
            g("conv_b")[perm].reshape(NDU, 128).T, f32),
        "xprojT": np.ascontiguousarray(
            g("xproj_w").T[perm].reshape(NDU, 128, 80).transpose(1, 0, 2)
            .reshape(128, -1).astype(f16)),
        "dtwT": np.ascontiguousarray(g("dt_w")[hs].T.astype(f16)),
        "dtb": np.ascontiguousarray(g("dt_b")[hs].reshape(NDH, 128).T, f32),
        "dvec": np.ascontiguousarray(g("D")[hs].reshape(NDH, 128).T, f32),
        "w2T": np.ascontiguousarray(
            (np.asarray(inp["proj_w"], f32)[:, d * DM:(d + 1) * DM]
             @ g("out_w")[:, hs]).T.reshape(NDH, 128, DM).transpose(1, 0, 2)
             .reshape(128, -1).astype(f16)),
        "ident": np.eye(128, dtype=f16),
    }


def _run(inp, rep=1, trace=False, variant="full"):
    nc = _get_program(rep, variant)
    in_maps = []
    for c in range(8):
        b, d, half = c >> 2, (c >> 1) & 1, c & 1
        in_maps.append(_prep_core_inputs(inp, b, d, half))
    return run_bass_kernel_spmd(nc, in_maps, list(range(8)), trace=trace)


def kernel(**inputs):
    res = _run(inputs, rep=1)
    x = np.asarray(inputs["x"], np.float32)
    proj_b = np.asarray(inputs["proj_b"], np.float32)
    out = np.empty((2, L, DM), np.float32)
    for b in range(2):
        acc = x[b] + proj_b
        for d in range(2):
            for half in range(2):
                c = (b << 2) | (d << 1) | half
                q = res.results[c]["q"].T          # (L, DM)
                if d == 1:
                    q = q[::-1]
                acc = acc + q
        out[b] = acc
    return out


if __name__ == "__main__":
    nc = _get_program(1)
    print("build ok")



# revision 2
# speedup vs baseline: 5.1378x; 5.1378x over previous
"""BiMamba adapter Trainium2 kernel, v2.

Sharding: 8 cores = (batch 2) x (direction 2) x (d_inner half 2), as v1.

Device program redesign vs v1:
- x arrives pre-transposed (f16): no on-chip transposes; LN stats via
  gpsimd partition_all_reduce; LN affine applied with 2 wide DVE ops.
- dt path: w = sigmoid(-(dt_lin+dt_b)) so dA_n = w^(n+1) (multiply
  ladder, no per-state exp) and dt = -ln(w) (sign folded into host-side
  B-row weights of xproj).
- scan phase broadcasts B/C rows with one gpsimd partition_broadcast
  each; the x6 chunk replication uses stride-0 access-pattern views.
- single 3D-AP DMAs for bulk loads/stores; f16 in/out; params packed
  into one 4KB block.
"""
import numpy as np

import concourse.bass as bass
import concourse.bacc as bacc
import concourse.tile as tile
from concourse import mybir
from concourse.bass_utils import run_bass_kernel_spmd
from concourse import bass_isa

F16 = mybir.dt.float16
F32 = mybir.dt.float32
OP = mybir.AluOpType
AF = mybir.ActivationFunctionType
RED = bass_isa.ReduceOp

L = 2048
DM = 768          # d_model
DI = 1536         # d_inner
DH = 768          # d_inner half per core
DTR = 48          # dt rank
NS = 16           # d_state
NDM = DM // 128   # 6
NDU = DI // 128   # 12
NDH = DH // 128   # 6
KC = 4            # conv taps
FC = 512          # psum free chunk
NFC = L // FC     # 4
WB = NDH * L      # 12288 own-half batched free size
WU = NDU * L      # 24576 full-u batched free size
UP = L + 4        # padded u chunk stride
WZ = DI + DH      # 2304 in_proj out channels

# packed param block column offsets
P_BU, P_BZ, P_CW, P_CB, P_DTBN, P_DV, P_EPS, P_END = 0, 12, 18, 66, 78, 84, 90, 91


def _build_program(rep=1, variant="cc"):
    nc = bacc.Bacc("TRN2", target_bir_lowering=False, debug=False, num_devices=8)

    def din(name, shape, dt=F16):
        return nc.dram_tensor(name, shape, dt, kind="ExternalInput").ap()

    aps = dict(
        xinT=din("xinT", [DM, L]),
        wuzT=din("wuzT", [DM, WZ]),
        pblk=din("pblk", [128, P_END], F32),
        xprojT=din("xprojT", [128, NDU * 80]),
        dtwT=din("dtwT", [DTR, DH]),
        w2T=din("w2T", [128, NDH * DM]),
        qout=nc.dram_tensor("q", [DM, L], F16, kind="ExternalOutput").ap(),
    )
    if variant.startswith("cc"):
        aps["xdpd"] = nc.dram_tensor("xdpd", [80, L], F32).ap()
        aps["xdsd"] = nc.dram_tensor("xdsd", [80, L], F32).ap()
    if variant == "cc2":
        aps["sgd"] = nc.dram_tensor("sgd", [128, WB], F16).ap()

    with tile.TileContext(nc) as tc:
        for _ in range(rep):
            _body(tc, nc, aps, variant)
    nc.compile()
    return nc


def _cv(t, cols=L):
    """View [128, n*cols] region as [128, n, cols]."""
    return t.rearrange("p (c l) -> p c l", l=cols)


def _bc(row_ap, n):
    """Broadcast [128, L] row-tile across n chunk views -> [128, n, L]."""
    return row_ap.rearrange("p (o l) -> p o l", o=1).to_broadcast([128, n, L])


def _wvec(vec_ap, n):
    """[128, n] per-chunk scalars -> [128, n, L] stride-0 view."""
    return vec_ap.rearrange("p (c o) -> p c o", o=1).to_broadcast([128, n, L])


def _body(tc, nc, aps, variant="full"):
    qout = aps["qout"]

    with tc.tile_pool(name="params", bufs=1) as pp, \
         tc.tile_pool(name="big", bufs=1) as bigp, \
         tc.tile_pool(name="mmps", bufs=4, space=bass.MemorySpace.PSUM) as mmps:

        pblk = pp.tile([128, P_END], F32, tag="pblk")
        nc.sync.dma_start(pblk[:], aps["pblk"])
        bu_sb = pblk[:, P_BU:P_BZ]
        bz_sb = pblk[:, P_BZ:P_CW]
        convw_sb = pblk[:, P_CW:P_CB]
        convb_sb = pblk[:, P_CB:P_DTBN]
        dtbn_sb = pblk[:, P_DTBN:P_DV]
        dvec_sb = pblk[:, P_DV:P_EPS]
        eps_sb = pblk[:, P_EPS:P_END]

        xdT = bigp.tile([80, L], F16, tag="xdT")
        if variant != "cc2":
            sg_big = bigp.tile([128, WB], F16, tag="sg_big")

        # ================= phase A =================
        pa_cm = tc.tile_pool(name="pa", bufs=1)
        pa = pa_cm.__enter__()
        xT = pa.tile([128, NDM * L], F16, tag="xT")
        nc.sync.dma_start(
            _cv(xT[:]), aps["xinT"].rearrange("(c p) l -> p c l", p=128))
        abc = pa.tile([128, L], F16, tag="abc")
        bbc0 = pa.tile([128, L], F16, tag="bbc0")

        # --- LN stats (over d_model = partitions x 6 chunks) ---
        with tc.tile_pool(name="st", bufs=1) as st:
            xsq = st.tile([128, NDM * L], F16, tag="xsq")
            nc.scalar.activation(xsq[:], xT[:], AF.Square)
            ssum = st.tile([128, L], F16, tag="ssum")
            qsum = st.tile([128, L], F16, tag="qsum")
            nc.vector.tensor_add(ssum[:], xT[:, 0:L], xT[:, L:2 * L])
            nc.vector.tensor_add(qsum[:], xsq[:, 0:L], xsq[:, L:2 * L])
            for c in range(2, NDM):
                nc.vector.tensor_add(ssum[:], ssum[:],
                                     xT[:, c * L:(c + 1) * L])
                nc.vector.tensor_add(qsum[:], qsum[:],
                                     xsq[:, c * L:(c + 1) * L])
            sred = st.tile([128, L], F32, tag="sred")
            qred = st.tile([128, L], F32, tag="qred")
            nc.gpsimd.partition_all_reduce(sred[:], ssum[:], 128, RED.add)
            nc.gpsimd.partition_all_reduce(qred[:], qsum[:], 128, RED.add)
            mu = st.tile([128, L], F32, tag="mu")
            nc.scalar.activation(mu[:], sred[:], AF.Copy, scale=1.0 / DM)
            msq = st.tile([128, L], F32, tag="msq")
            nc.scalar.activation(msq[:], mu[:], AF.Square)
            nc.vector.scalar_tensor_tensor(qred[:], qred[:], 1.0 / DM,
                                           msq[:], OP.mult, OP.subtract)
            nc.scalar.activation(msq[:], qred[:], AF.Sqrt, bias=eps_sb)
            with nc.allow_low_precision(reason="rstd ~O(1), f16 ok"):
                nc.vector.reciprocal(abc[:], msq[:])
            nc.vector.scalar_tensor_tensor(bbc0[:], mu[:], -1.0,
                                           abc[:], OP.mult, OP.mult)
        # hT = xT * a + b   (in place on xT)
        nc.vector.tensor_tensor(_cv(xT[:]), _cv(xT[:]),
                                _bc(abc[:], NDM), OP.mult)
        nc.vector.tensor_tensor(_cv(xT[:]), _cv(xT[:]),
                                _bc(bbc0[:], NDM), OP.add)

        # --- in_proj ---
        NU = NDH if variant.startswith("cc") else NDU   # u chunks computed locally
        ubp_cm = tc.tile_pool(name="ub", bufs=1)
        ubp = ubp_cm.__enter__()
        if variant == "cc2":
            sg_big = ubp.tile([128, WB], F16, tag="sg_big", name="sg_big")
        u_big = ubp.tile([128, NU * UP], F16, tag="u_big")
        nc.vector.memset(_cv(u_big[:], UP)[:, :, 0:4], 0.0)
        with tc.tile_pool(name="wch", bufs=1) as wchp, \
             tc.tile_pool(name="ipps", bufs=2,
                          space=bass.MemorySpace.PSUM) as ipps:
            wuz = wchp.tile([128, NDM * WZ], F16, tag="wuz")
            nc.sync.dma_start(
                _cv(wuz[:], WZ),
                aps["wuzT"].rearrange("(c p) n -> p c n", p=128))
            ics = list(range(NU)) + list(range(NDU, NDU + NDH))
            for ic in ics:
                is_u = ic < NDU
                ps = ipps.tile([128, L], F32, tag="mm", name="ps")
                for fc in range(NFC):
                    for mc in range(NDM):
                        nc.tensor.matmul(
                            ps[:, fc * FC:(fc + 1) * FC],
                            wuz[:, mc * WZ + ic * 128:mc * WZ + (ic + 1) * 128],
                            xT[:, mc * L + fc * FC:mc * L + (fc + 1) * FC],
                            start=(mc == 0), stop=(mc == NDM - 1))
                if is_u:
                    nc.scalar.activation(
                        u_big[:, ic * UP + 4:ic * UP + 4 + L],
                        ps[:], AF.Identity, bias=bu_sb[:, ic:ic + 1])
                else:
                    zc = ic - NDU
                    nc.scalar.activation(
                        sg_big[:, zc * L:(zc + 1) * L],
                        ps[:], AF.Silu, bias=bz_sb[:, zc:zc + 1])
            if variant == "mm2x":
                scr2 = wchp.tile([128, L], F16, tag="scr2")
                for ic in range(NDU + NDH):
                    ps = ipps.tile([128, L], F32, tag="mm", name="ps")
                    for fc in range(NFC):
                        for mc in range(NDM):
                            nc.tensor.matmul(
                                ps[:, fc * FC:(fc + 1) * FC],
                                wuz[:, mc * WZ + ic * 128:
                                    mc * WZ + (ic + 1) * 128],
                                xT[:, mc * L + fc * FC:mc * L + (fc + 1) * FC],
                                start=(mc == 0), stop=(mc == NDM - 1))
                    nc.scalar.activation(scr2[:], ps[:], AF.Identity,
                                         bias=bu_sb[:, 0:1])

        if variant == "cc2":
            nc.sync.dma_start(aps["sgd"], sg_big[:])   # spill gate to DRAM

        # --- conv: per-chunk fused stt taps, silu in place ---
        cvp_cm = tc.tile_pool(name="cvpool", bufs=1, side="right")
        cvp = cvp_cm.__enter__()
        u_sc = cvp.tile([128, NU * L], F16, tag="u_sc")
        for ic in range(NU):
            dst = u_sc[:, ic * L:(ic + 1) * L]
            nc.vector.tensor_scalar_mul(
                dst, u_big[:, ic * UP + 1:ic * UP + 1 + L],
                convw_sb[:, ic * KC:ic * KC + 1])
            for k in range(1, KC):
                nc.vector.scalar_tensor_tensor(
                    dst, u_big[:, ic * UP + 1 + k:ic * UP + 1 + k + L],
                    convw_sb[:, ic * KC + k:ic * KC + k + 1],
                    dst, OP.mult, OP.add)
            nc.scalar.activation(dst, dst, AF.Silu,
                                 bias=convb_sb[:, ic:ic + 1])

        ubp_cm.__exit__(None, None, None)     # u_big freed
        pa_cm.__exit__(None, None, None)      # xT freed

        # --- xproj / dt / v / yacc ---
        m1_cm = tc.tile_pool(name="m1", bufs=1)
        m1 = m1_cm.__enter__()
        w_big = m1.tile([128, WB], F16, tag="w_big")
        v_big = m1.tile([128, WB], F16, tag="v_big")
        yacc = m1.tile([128, WB], F16, tag="yacc")
        with tc.tile_pool(name="xp", bufs=1) as xpp:
            with tc.tile_pool(name="xps", bufs=2,
                              space=bass.MemorySpace.PSUM) as xps:
                xprojT_sb = xpp.tile([128, NU * 80], F16, tag="xprojT")
                nc.sync.dma_start(xprojT_sb[:],
                                  aps["xprojT"][:, 0:NU * 80])
                if variant.startswith("cc"):
                    xq_sb = xpp.tile([80, L], F32, tag="xq", name="xq_sb")
                else:
                    xq_sb = None
                for fc in range(NFC):
                    psx = xps.tile([80, FC], F32, tag="psx", name="psx")
                    for kc in range(NU):
                        nc.tensor.matmul(
                            psx[:], xprojT_sb[:, kc * 80:(kc + 1) * 80],
                            u_sc[:, kc * L + fc * FC:kc * L + (fc + 1) * FC],
                            start=(kc == 0), stop=(kc == NU - 1))
                    if variant.startswith("cc"):
                        nc.scalar.activation(xq_sb[:, fc * FC:(fc + 1) * FC],
                                             psx[:], AF.Copy)
                    else:
                        nc.scalar.activation(xdT[:, fc * FC:(fc + 1) * FC],
                                             psx[:], AF.Copy)
                if variant.startswith("cc"):
                    nc.sync.dma_start(aps["xdpd"], xq_sb[:])
                    nc.gpsimd.collective_compute(
                        "AllReduce", OP.add,
                        replica_groups=[[0, 1], [2, 3], [4, 5], [6, 7]],
                        ins=[aps["xdpd"]], outs=[aps["xdsd"]])
                    nc.gpsimd.dma_start(xdT[:], aps["xdsd"])

            # --- dt -> w = sigmoid(-(dt_lin + dt_b)) ---
            dtw_sb = xpp.tile([DTR, DH], F16, tag="dtw")
            nc.sync.dma_start(dtw_sb[:], aps["dtwT"])
            with tc.tile_pool(name="dtps", bufs=2,
                              space=bass.MemorySpace.PSUM) as dtps:
                for mc in range(NDH):
                    ps = dtps.tile([128, L], F32, tag="dt", name="ps")
                    for fc in range(NFC):
                        nc.tensor.matmul(
                            ps[:, fc * FC:(fc + 1) * FC],
                            dtw_sb[:, mc * 128:(mc + 1) * 128],
                            xdT[0:DTR, fc * FC:(fc + 1) * FC],
                            start=True, stop=True)
                    nc.scalar.activation(
                        w_big[:, mc * L:(mc + 1) * L],
                        ps[:], AF.Sigmoid, scale=-1.0,
                        bias=dtbn_sb[:, mc:mc + 1])

        # v = (-ln w) * u, sign folded into host B rows; yacc = u*D
        nc.scalar.activation(v_big[:], w_big[:], AF.Ln)
        nc.vector.tensor_mul(v_big[:], v_big[:], u_sc[:, 0:WB])
        nc.vector.tensor_tensor(_cv(yacc[:]), _cv(u_sc[:, 0:WB]),
                                _wvec(dvec_sb, NDH), OP.mult)
        cvp_cm.__exit__(None, None, None)     # u_sc freed

        # poison w at each chunk's first column: dA -> 0 resets the
        # batched scan exactly (v was computed from unpoisoned w).
        nc.vector.memset(_cv(w_big[:])[:, :, 0:1], 0.0)

        # ================= phase B: scan =================
        with tc.tile_pool(name="bt", bufs=1) as btp:
            at_big = btp.tile([128, WB], F16, tag="at_big")
            tmp1 = btp.tile([128, WB], F16, tag="tmp1")
            tmp2 = btp.tile([128, WB], F16, tag="tmp2")
            with tc.tile_pool(name="rows",
                              bufs=(1 if variant == "cc2" else 2)) as rowp:
                if variant == "nob":
                    bbc0_t = rowp.tile([128, L], F16, tag="bbc", name="bbc")
                    cbc0_t = rowp.tile([128, L], F16, tag="cbc", name="cbc")
                    nc.vector.memset(bbc0_t[:], 0.25)
                    nc.vector.memset(cbc0_t[:], 0.25)
                xdsd_flat = (aps["xdsd"].rearrange("r l -> (r l)")
                             if variant.startswith("cc") else None)
                bbc2 = cbc2 = None
                for n in range(NS if variant != "noscan" else 0):
                    if variant == "nob":
                        bbc, cbc = bbc0_t, cbc0_t
                    elif variant == "cc2":
                        if n % 2 == 0:
                            bbc2 = rowp.tile([128, 2 * L], F16, tag="bbc",
                                             name="bbc2")
                            cbc2 = rowp.tile([128, 2 * L], F16, tag="cbc",
                                             name="cbc2")
                            brow = rowp.tile([1, 2 * L], F16, tag="brow",
                                             name="brow")
                            crow = rowp.tile([1, 2 * L], F16, tag="crow",
                                             name="crow")
                            nc.gpsimd.dma_start(
                                brow[0:1, :],
                                xdsd_flat[(DTR + n) * L:(DTR + n + 2) * L])
                            nc.gpsimd.dma_start(
                                crow[0:1, :],
                                xdsd_flat[(DTR + NS + n) * L:
                                          (DTR + NS + n + 2) * L])
                            nc.gpsimd.partition_broadcast(bbc2[:],
                                                          brow[0:1, :])
                            nc.gpsimd.partition_broadcast(cbc2[:],
                                                          crow[0:1, :])
                        off = (n % 2) * L
                        bbc = bbc2[:, off:off + L]
                        cbc = cbc2[:, off:off + L]
                    else:
                        bbc = rowp.tile([128, L], F16, tag="bbc", name="bbc")
                        cbc = rowp.tile([128, L], F16, tag="cbc", name="cbc")
                        brow = rowp.tile([1, L], F16, tag="brow", name="brow")
                        crow = rowp.tile([1, L], F16, tag="crow", name="crow")
                        nc.sync.dma_start(brow[0:1, :],
                                          xdT[DTR + n:DTR + n + 1, :])
                        nc.sync.dma_start(crow[0:1, :],
                                          xdT[DTR + NS + n:DTR + NS + n + 1, :])
                        nc.gpsimd.partition_broadcast(bbc[:], brow[0:1, :])
                        nc.gpsimd.partition_broadcast(cbc[:], crow[0:1, :])
                    at = w_big if n == 0 else at_big
                    if variant != "nomul":
                        nc.vector.tensor_tensor(_cv(tmp1[:]), _cv(v_big[:]),
                                                _bc(bbc[:], NDH), OP.mult)
                        sc_in = tmp1
                    else:
                        sc_in = v_big
                    if variant == "nos":
                        nc.vector.tensor_tensor(tmp2[:], at[:], sc_in[:],
                                                OP.mult)
                    else:
                        nc.vector.tensor_tensor_scan(tmp2[:], at[:], sc_in[:],
                                                     0.0, OP.mult, OP.add)
                    if variant != "nomul":
                        nc.vector.tensor_tensor(_cv(tmp1[:]), _cv(tmp2[:]),
                                                _bc(cbc[:], NDH), OP.mult)
                        nc.vector.tensor_add(yacc[:], yacc[:], tmp1[:])
                    if n < NS - 1 and variant != "nolad":
                        nc.vector.tensor_mul(at_big[:], at[:], w_big[:])

            # ============ phase C: gate + out_proj ============
            if variant == "cc2":
                nc.sync.dma_start(tmp2[:], aps["sgd"])
                nc.vector.tensor_mul(yacc[:], yacc[:], tmp2[:])
            else:
                nc.vector.tensor_mul(yacc[:], yacc[:], sg_big[:])
            with tc.tile_pool(name="w2", bufs=1) as w2p, \
                 tc.tile_pool(name="cps", bufs=2,
                              space=bass.MemorySpace.PSUM) as cps:
                w2_sb = w2p.tile([128, NDH * DM], F16, tag="w2T")
                nc.sync.dma_start(w2_sb[:], aps["w2T"])
                q_big = btp.tile([128, NDM * L], F16, tag="tmp1")
                for mc in range(NDM):
                    ps = cps.tile([128, L], F32, tag="psc", name="psc")
                    for fc in range(NFC):
                        for kc in range(NDH):
                            nc.tensor.matmul(
                                ps[:, fc * FC:(fc + 1) * FC],
                                w2_sb[:, kc * DM + mc * 128:
                                      kc * DM + (mc + 1) * 128],
                                yacc[:, kc * L + fc * FC:
                                     kc * L + (fc + 1) * FC],
                                start=(kc == 0), stop=(kc == NDH - 1))
                    nc.scalar.activation(q_big[:, mc * L:(mc + 1) * L],
                                         ps[:], AF.Copy)
                nc.sync.dma_start(
                    qout.rearrange("(c p) l -> p c l", p=128),
                    _cv(q_big[:]))
        m1_cm.__exit__(None, None, None)


_CACHE = {}


def _get_program(rep=1, variant="cc"):
    key = (rep, variant)
    if key not in _CACHE:
        _CACHE[key] = _build_program(rep, variant)
    return _CACHE[key]


def _prep_core_inputs(inp, b, d, half):
    f32 = np.float32
    f16 = np.float16
    pref = "mf" if d == 0 else "mb"
    g = lambda k: np.asarray(inp[f"{pref}_{k}"], f32)
    ln_w = np.asarray(inp["ln_w"], f32)
    ln_b = np.asarray(inp["ln_b"], f32)
    in_w = g("in_w")
    x = np.asarray(inp["x"], f32)[b]
    if d == 1:
        x = x[::-1]
    perm = np.concatenate([np.arange(half * DH, (half + 1) * DH),
                           np.arange((1 - half) * DH, (2 - half) * DH)])
    hs = slice(half * DH, (half + 1) * DH)
    wu = in_w[0:DI][perm]
    wz = in_w[DI + half * DH:DI + (half + 1) * DH]
    A = -np.exp(g("A_log")[hs])
    assert np.abs(A + np.arange(1, NS + 1)).max() < 1e-4, \
        "kernel assumes A[:, n] == -(n+1)"
    xproj = g("xproj_w").T[perm].copy()        # (DI, 80) permuted
    xproj[:, DTR:DTR + NS] *= -1.0             # fold v = -ln(w)*u sign into B

    pb = np.zeros((128, P_END), f32)
    pb[:, P_BU:P_BZ] = (wu @ ln_b).reshape(NDU, 128).T
    pb[:, P_BZ:P_CW] = (wz @ ln_b).reshape(NDH, 128).T
    pb[:, P_CW:P_CB] = (g("conv_w")[perm].reshape(NDU, 128, KC)
                        .transpose(1, 0, 2).reshape(128, -1))
    pb[:, P_CB:P_DTBN] = g("conv_b")[perm].reshape(NDU, 128).T
    pb[:, P_DTBN:P_DV] = -g("dt_b")[hs].reshape(NDH, 128).T
    pb[:, P_DV:P_EPS] = g("D")[hs].reshape(NDH, 128).T
    pb[:, P_EPS:P_END] = 1e-5
    return {
        "xinT": np.ascontiguousarray(x.T.astype(f16)),
        "wuzT": np.ascontiguousarray(
            (np.concatenate([wu.T, wz.T], axis=1)
             * ln_w[:, None]).astype(f16)),
        "pblk": np.ascontiguousarray(pb),
        "xprojT": np.ascontiguousarray(
            xproj.reshape(NDU, 128, 80).transpose(1, 0, 2)
            .reshape(128, -1).astype(f16)),
        "dtwT": np.ascontiguousarray(g("dt_w")[hs].T.astype(f16)),
        "w2T": np.ascontiguousarray(
            (np.asarray(inp["proj_w"], f32)[:, d * DM:(d + 1) * DM]
             @ g("out_w")[:, hs]).T.reshape(NDH, 128, DM).transpose(1, 0, 2)
             .reshape(128, -1).astype(f16)),
    }


def _run(inp, rep=1, trace=False, variant="cc"):
    nc = _get_program(rep, variant)
    in_maps = []
    for c in range(8):
        b, d, half = c >> 2, (c >> 1) & 1, c & 1
        in_maps.append(_prep_core_inputs(inp, b, d, half))
    return run_bass_kernel_spmd(nc, in_maps, list(range(8)), trace=trace)


def kernel(**inputs):
    res = _run(inputs, rep=1)
    x = np.asarray(inputs["x"], np.float32)
    proj_b = np.asarray(inputs["proj_b"], np.float32)
    out = np.empty((2, L, DM), np.float32)
    for b in range(2):
        acc = x[b] + proj_b
        for d in range(2):
            for half in range(2):
                c = (b << 2) | (d << 1) | half
                q = res.results[c]["q"].astype(np.float32).T   # (L, DM)
                if d == 1:
                    q = q[::-1]
                acc = acc + q
        out[b] = acc
    return out


if __name__ == "__main__":
    nc = _get_program(1)
    print("build ok")


# revision 3
# speedup vs baseline: 5.9996x; 1.1678x over previous
"""BiMamba adapter Trainium2 kernel, v2.

Sharding: 8 cores = (batch 2) x (direction 2) x (d_inner half 2), as v1.

Device program redesign vs v1:
- x arrives pre-transposed (f16): no on-chip transposes; LN stats via
  gpsimd partition_all_reduce; LN affine applied with 2 wide DVE ops.
- dt path: w = sigmoid(-(dt_lin+dt_b)) so dA_n = w^(n+1) (multiply
  ladder, no per-state exp) and dt = -ln(w) (sign folded into host-side
  B-row weights of xproj).
- scan phase broadcasts B/C rows with one gpsimd partition_broadcast
  each; the x6 chunk replication uses stride-0 access-pattern views.
- single 3D-AP DMAs for bulk loads/stores; f16 in/out; params packed
  into one 4KB block.
"""
import numpy as np

import concourse.bass as bass
import concourse.bacc as bacc
import concourse.tile as tile
from concourse import mybir
from concourse.bass_utils import run_bass_kernel_spmd
from concourse import bass_isa

F16 = mybir.dt.float16
F32 = mybir.dt.float32
OP = mybir.AluOpType
AF = mybir.ActivationFunctionType
RED = bass_isa.ReduceOp

L = 2048
DM = 768          # d_model
DI = 1536         # d_inner
DH = 768          # d_inner half per core
DTR = 48          # dt rank
NS = 16           # d_state
NDM = DM // 128   # 6
NDU = DI // 128   # 12
NDH = DH // 128   # 6
KC = 4            # conv taps
FC = 512          # psum free chunk
NFC = L // FC     # 4
WB = NDH * L      # 12288 own-half batched free size
WU = NDU * L      # 24576 full-u batched free size
UP = L + 4        # padded u chunk stride
WZ = DI + DH      # 2304 in_proj out channels

# packed param block column offsets
P_BU, P_BZ, P_CW, P_CB, P_DTBN, P_DV, P_EPS, P_END = 0, 12, 18, 66, 78, 84, 90, 91


def _build_program(rep=1, variant="cc"):
    nc = bacc.Bacc("TRN2", target_bir_lowering=False, debug=False, num_devices=8)

    def din(name, shape, dt=F16):
        return nc.dram_tensor(name, shape, dt, kind="ExternalInput").ap()

    aps = dict(
        xinT=din("xinT", [DM, L]),
        wuzT=din("wuzT", [DM, WZ]),
        pblk=din("pblk", [128, P_END], F32),
        xprojT=din("xprojT", [128, NDU * 80]),
        dtwT=din("dtwT", [DTR, DH]),
        w2T=din("w2T", [128, NDH * DM]),
        qout=nc.dram_tensor("q", [DM, L], F16, kind="ExternalOutput").ap(),
    )
    if variant.startswith("cc"):
        aps["xdpd"] = nc.dram_tensor("xdpd", [80, L], F32).ap()
        aps["xdsd"] = nc.dram_tensor("xdsd", [80, L], F32).ap()
    if variant == "cc2":
        aps["sgd"] = nc.dram_tensor("sgd", [128, WB], F16).ap()

    with tile.TileContext(nc) as tc:
        for _ in range(rep):
            _body(tc, nc, aps, variant)
    nc.compile()
    return nc


def _cv(t, cols=L):
    """View [128, n*cols] region as [128, n, cols]."""
    return t.rearrange("p (c l) -> p c l", l=cols)


def _bc(row_ap, n):
    """Broadcast [128, L] row-tile across n chunk views -> [128, n, L]."""
    return row_ap.rearrange("p (o l) -> p o l", o=1).to_broadcast([128, n, L])


def _wvec(vec_ap, n):
    """[128, n] per-chunk scalars -> [128, n, L] stride-0 view."""
    return vec_ap.rearrange("p (c o) -> p c o", o=1).to_broadcast([128, n, L])


def _body(tc, nc, aps, variant="full"):
    qout = aps["qout"]

    with tc.tile_pool(name="params", bufs=1) as pp, \
         tc.tile_pool(name="big", bufs=1) as bigp, \
         tc.tile_pool(name="mmps", bufs=4, space=bass.MemorySpace.PSUM) as mmps:

        pblk = pp.tile([128, P_END], F32, tag="pblk")
        nc.sync.dma_start(pblk[:], aps["pblk"])
        bu_sb = pblk[:, P_BU:P_BZ]
        bz_sb = pblk[:, P_BZ:P_CW]
        convw_sb = pblk[:, P_CW:P_CB]
        convb_sb = pblk[:, P_CB:P_DTBN]
        dtbn_sb = pblk[:, P_DTBN:P_DV]
        dvec_sb = pblk[:, P_DV:P_EPS]
        eps_sb = pblk[:, P_EPS:P_END]

        xdT = bigp.tile([80, L], F16, tag="xdT")
        if variant != "cc2":
            sg_big = bigp.tile([128, WB], F16, tag="sg_big")

        # ================= phase A =================
        pa_cm = tc.tile_pool(name="pa", bufs=1)
        pa = pa_cm.__enter__()
        xT = pa.tile([128, NDM * L], F16, tag="xT")
        nc.sync.dma_start(
            _cv(xT[:]), aps["xinT"].rearrange("(c p) l -> p c l", p=128))
        abc = pa.tile([128, L], F16, tag="abc")
        bbc0 = pa.tile([128, L], F16, tag="bbc0")

        # --- LN stats (over d_model = partitions x 6 chunks) ---
        with tc.tile_pool(name="st", bufs=1) as st:
            xsq = st.tile([128, NDM * L], F16, tag="xsq")
            nc.scalar.activation(xsq[:], xT[:], AF.Square)
            ssum = st.tile([128, L], F16, tag="ssum")
            qsum = st.tile([128, L], F16, tag="qsum")
            nc.vector.tensor_add(ssum[:], xT[:, 0:L], xT[:, L:2 * L])
            nc.vector.tensor_add(qsum[:], xsq[:, 0:L], xsq[:, L:2 * L])
            for c in range(2, NDM):
                nc.vector.tensor_add(ssum[:], ssum[:],
                                     xT[:, c * L:(c + 1) * L])
                nc.vector.tensor_add(qsum[:], qsum[:],
                                     xsq[:, c * L:(c + 1) * L])
            sred = st.tile([128, L], F32, tag="sred")
            qred = st.tile([128, L], F32, tag="qred")
            nc.gpsimd.partition_all_reduce(sred[:], ssum[:], 128, RED.add)
            nc.gpsimd.partition_all_reduce(qred[:], qsum[:], 128, RED.add)
            mu = st.tile([128, L], F32, tag="mu")
            nc.scalar.activation(mu[:], sred[:], AF.Copy, scale=1.0 / DM)
            msq = st.tile([128, L], F32, tag="msq")
            nc.scalar.activation(msq[:], mu[:], AF.Square)
            nc.vector.scalar_tensor_tensor(qred[:], qred[:], 1.0 / DM,
                                           msq[:], OP.mult, OP.subtract)
            nc.scalar.activation(msq[:], qred[:], AF.Sqrt, bias=eps_sb)
            with nc.allow_low_precision(reason="rstd ~O(1), f16 ok"):
                nc.vector.reciprocal(abc[:], msq[:])
            nc.vector.scalar_tensor_tensor(bbc0[:], mu[:], -1.0,
                                           abc[:], OP.mult, OP.mult)
        # hT = xT * a + b   (in place on xT)
        nc.vector.tensor_tensor(_cv(xT[:]), _cv(xT[:]),
                                _bc(abc[:], NDM), OP.mult)
        nc.vector.tensor_tensor(_cv(xT[:]), _cv(xT[:]),
                                _bc(bbc0[:], NDM), OP.add)

        # --- in_proj ---
        NU = NDH if variant.startswith("cc") else NDU   # u chunks computed locally
        ubp_cm = tc.tile_pool(name="ub", bufs=1)
        ubp = ubp_cm.__enter__()
        if variant == "cc2":
            sg_big = ubp.tile([128, WB], F16, tag="sg_big", name="sg_big")
        u_big = ubp.tile([128, NU * UP], F16, tag="u_big")
        nc.vector.memset(_cv(u_big[:], UP)[:, :, 0:4], 0.0)
        with tc.tile_pool(name="wch", bufs=1) as wchp, \
             tc.tile_pool(name="ipps", bufs=2,
                          space=bass.MemorySpace.PSUM) as ipps:
            wuz = wchp.tile([128, NDM * WZ], F16, tag="wuz")
            nc.sync.dma_start(
                _cv(wuz[:], WZ),
                aps["wuzT"].rearrange("(c p) n -> p c n", p=128))
            ics = list(range(NU)) + list(range(NDU, NDU + NDH))
            for ic in ics:
                is_u = ic < NDU
                ps = ipps.tile([128, L], F32, tag="mm", name="ps")
                for fc in range(NFC):
                    for mc in range(NDM):
                        nc.tensor.matmul(
                            ps[:, fc * FC:(fc + 1) * FC],
                            wuz[:, mc * WZ + ic * 128:mc * WZ + (ic + 1) * 128],
                            xT[:, mc * L + fc * FC:mc * L + (fc + 1) * FC],
                            start=(mc == 0), stop=(mc == NDM - 1))
                if is_u:
                    nc.scalar.activation(
                        u_big[:, ic * UP + 4:ic * UP + 4 + L],
                        ps[:], AF.Identity, bias=bu_sb[:, ic:ic + 1])
                else:
                    zc = ic - NDU
                    nc.scalar.activation(
                        sg_big[:, zc * L:(zc + 1) * L],
                        ps[:], AF.Silu, bias=bz_sb[:, zc:zc + 1])
            if variant == "mm2x":
                scr2 = wchp.tile([128, L], F16, tag="scr2")
                for ic in range(NDU + NDH):
                    ps = ipps.tile([128, L], F32, tag="mm", name="ps")
                    for fc in range(NFC):
                        for mc in range(NDM):
                            nc.tensor.matmul(
                                ps[:, fc * FC:(fc + 1) * FC],
                                wuz[:, mc * WZ + ic * 128:
                                    mc * WZ + (ic + 1) * 128],
                                xT[:, mc * L + fc * FC:mc * L + (fc + 1) * FC],
                                start=(mc == 0), stop=(mc == NDM - 1))
                    nc.scalar.activation(scr2[:], ps[:], AF.Identity,
                                         bias=bu_sb[:, 0:1])

        if variant == "cc2":
            nc.sync.dma_start(aps["sgd"], sg_big[:])   # spill gate to DRAM

        # --- conv: per-chunk fused stt taps, silu in place ---
        cvp_cm = tc.tile_pool(name="cvpool", bufs=1, side="right")
        cvp = cvp_cm.__enter__()
        u_sc = cvp.tile([128, NU * L], F16, tag="u_sc")
        for ic in range(NU):
            dst = u_sc[:, ic * L:(ic + 1) * L]
            nc.vector.tensor_scalar_mul(
                dst, u_big[:, ic * UP + 1:ic * UP + 1 + L],
                convw_sb[:, ic * KC:ic * KC + 1])
            for k in range(1, KC):
                nc.vector.scalar_tensor_tensor(
                    dst, u_big[:, ic * UP + 1 + k:ic * UP + 1 + k + L],
                    convw_sb[:, ic * KC + k:ic * KC + k + 1],
                    dst, OP.mult, OP.add)
            nc.scalar.activation(dst, dst, AF.Silu,
                                 bias=convb_sb[:, ic:ic + 1])

        ubp_cm.__exit__(None, None, None)     # u_big freed
        pa_cm.__exit__(None, None, None)      # xT freed

        # --- xproj / dt / v / yacc ---
        m1_cm = tc.tile_pool(name="m1", bufs=1)
        m1 = m1_cm.__enter__()
        w_big = m1.tile([128, WB], F16, tag="w_big")
        v_big = m1.tile([128, WB], F16, tag="v_big")
        yacc = m1.tile([128, WB], F16, tag="yacc")
        with tc.tile_pool(name="xp", bufs=1) as xpp:
            with tc.tile_pool(name="xps", bufs=2,
                              space=bass.MemorySpace.PSUM) as xps:
                xprojT_sb = xpp.tile([128, NU * 80], F16, tag="xprojT")
                nc.sync.dma_start(xprojT_sb[:],
                                  aps["xprojT"][:, 0:NU * 80])
                if variant.startswith("cc"):
                    xq_sb = xpp.tile([80, L], F32, tag="xq", name="xq_sb")
                else:
                    xq_sb = None
                for fc in range(NFC):
                    psx = xps.tile([80, FC], F32, tag="psx", name="psx")
                    for kc in range(NU):
                        nc.tensor.matmul(
                            psx[:], xprojT_sb[:, kc * 80:(kc + 1) * 80],
                            u_sc[:, kc * L + fc * FC:kc * L + (fc + 1) * FC],
                            start=(kc == 0), stop=(kc == NU - 1))
                    if variant.startswith("cc"):
                        nc.scalar.activation(xq_sb[:, fc * FC:(fc + 1) * FC],
                                             psx[:], AF.Copy)
                    else:
                        nc.scalar.activation(xdT[:, fc * FC:(fc + 1) * FC],
                                             psx[:], AF.Copy)
                if variant.startswith("cc"):
                    nc.sync.dma_start(aps["xdpd"], xq_sb[:])
                    nc.gpsimd.collective_compute(
                        "AllReduce", OP.add,
                        replica_groups=[[0, 1], [2, 3], [4, 5], [6, 7]],
                        ins=[aps["xdpd"]], outs=[aps["xdsd"]])
                    nc.gpsimd.dma_start(xdT[:], aps["xdsd"])

            # --- dt -> w = sigmoid(-(dt_lin + dt_b)) ---
            dtw_sb = xpp.tile([DTR, DH], F16, tag="dtw")
            nc.sync.dma_start(dtw_sb[:], aps["dtwT"])
            with tc.tile_pool(name="dtps", bufs=2,
                              space=bass.MemorySpace.PSUM) as dtps:
                for mc in range(NDH):
                    ps = dtps.tile([128, L], F32, tag="dt", name="ps")
                    for fc in range(NFC):
                        nc.tensor.matmul(
                            ps[:, fc * FC:(fc + 1) * FC],
                            dtw_sb[:, mc * 128:(mc + 1) * 128],
                            xdT[0:DTR, fc * FC:(fc + 1) * FC],
                            start=True, stop=True)
                    nc.scalar.activation(
                        w_big[:, mc * L:(mc + 1) * L],
                        ps[:], AF.Sigmoid, scale=-1.0,
                        bias=dtbn_sb[:, mc:mc + 1])

        # v = (-ln w) * u, sign folded into host B rows; yacc = u*D
        nc.scalar.activation(v_big[:], w_big[:], AF.Ln)
        nc.vector.tensor_mul(v_big[:], v_big[:], u_sc[:, 0:WB])
        nc.vector.tensor_tensor(_cv(yacc[:]), _cv(u_sc[:, 0:WB]),
                                _wvec(dvec_sb, NDH), OP.mult)
        cvp_cm.__exit__(None, None, None)     # u_sc freed

        # poison w at each chunk's first column: dA -> 0 resets the
        # batched scan exactly (v was computed from unpoisoned w).
        nc.vector.memset(_cv(w_big[:])[:, :, 0:1], 0.0)

        # ================= phase B: scan =================
        with tc.tile_pool(name="bt", bufs=1) as btp:
            at_big = btp.tile([128, WB], F16, tag="at_big")
            tmp1 = btp.tile([128, WB], F16, tag="tmp1")
            tmp2 = btp.tile([128, WB], F16, tag="tmp2")
            with tc.tile_pool(name="rows",
                              bufs=(1 if variant == "cc2" else 2)) as rowp:
                if variant == "nob":
                    bbc0_t = rowp.tile([128, L], F16, tag="bbc", name="bbc")
                    cbc0_t = rowp.tile([128, L], F16, tag="cbc", name="cbc")
                    nc.vector.memset(bbc0_t[:], 0.25)
                    nc.vector.memset(cbc0_t[:], 0.25)
                xdsd_flat = (aps["xdsd"].rearrange("r l -> (r l)")
                             if variant.startswith("cc") else None)
                bbc2 = cbc2 = None
                for n in range(NS if variant != "noscan" else 0):
                    if variant == "nob":
                        bbc, cbc = bbc0_t, cbc0_t
                    elif variant == "cc2":
                        if n % 2 == 0:
                            bbc2 = rowp.tile([128, 2 * L], F16, tag="bbc",
                                             name="bbc2")
                            cbc2 = rowp.tile([128, 2 * L], F16, tag="cbc",
                                             name="cbc2")
                            brow = rowp.tile([1, 2 * L], F16, tag="brow",
                                             name="brow")
                            crow = rowp.tile([1, 2 * L], F16, tag="crow",
                                             name="crow")
                            nc.gpsimd.dma_start(
                                brow[0:1, :],
                                xdsd_flat[(DTR + n) * L:(DTR + n + 2) * L])
                            nc.gpsimd.dma_start(
                                crow[0:1, :],
                                xdsd_flat[(DTR + NS + n) * L:
                                          (DTR + NS + n + 2) * L])
                            nc.gpsimd.partition_broadcast(bbc2[:],
                                                          brow[0:1, :])
                            nc.gpsimd.partition_broadcast(cbc2[:],
                                                          crow[0:1, :])
                        off = (n % 2) * L
                        bbc = bbc2[:, off:off + L]
                        cbc = cbc2[:, off:off + L]
                    else:
                        bbc = rowp.tile([128, L], F16, tag="bbc", name="bbc")
                        cbc = rowp.tile([128, L], F16, tag="cbc", name="cbc")
                        brow = rowp.tile([1, L], F16, tag="brow", name="brow")
                        crow = rowp.tile([1, L], F16, tag="crow", name="crow")
                        nc.sync.dma_start(brow[0:1, :],
                                          xdT[DTR + n:DTR + n + 1, :])
                        nc.sync.dma_start(crow[0:1, :],
                                          xdT[DTR + NS + n:DTR + NS + n + 1, :])
                        nc.gpsimd.partition_broadcast(bbc[:], brow[0:1, :])
                        nc.gpsimd.partition_broadcast(cbc[:], crow[0:1, :])
                    at = w_big if n == 0 else at_big
                    if variant != "nomul":
                        nc.vector.tensor_tensor(_cv(tmp1[:]), _cv(v_big[:]),
                                                _bc(bbc[:], NDH), OP.mult)
                        sc_in = tmp1
                    else:
                        sc_in = v_big
                    if variant == "nos":
                        nc.vector.tensor_tensor(tmp2[:], at[:], sc_in[:],
                                                OP.mult)
                    else:
                        nc.vector.tensor_tensor_scan(tmp2[:], at[:], sc_in[:],
                                                     0.0, OP.mult, OP.add)
                    if variant != "nomul":
                        nc.vector.tensor_tensor(_cv(tmp1[:]), _cv(tmp2[:]),
                                                _bc(cbc[:], NDH), OP.mult)
                        nc.vector.tensor_add(yacc[:], yacc[:], tmp1[:])
                    if n < NS - 1 and variant != "nolad":
                        nc.vector.tensor_mul(at_big[:], at[:], w_big[:])

            # ============ phase C: gate + out_proj ============
            if variant == "cc2":
                nc.sync.dma_start(tmp2[:], aps["sgd"])
                nc.vector.tensor_mul(yacc[:], yacc[:], tmp2[:])
            else:
                nc.vector.tensor_mul(yacc[:], yacc[:], sg_big[:])
            with tc.tile_pool(name="w2", bufs=1) as w2p, \
                 tc.tile_pool(name="cps", bufs=2,
                              space=bass.MemorySpace.PSUM) as cps:
                w2_sb = w2p.tile([128, NDH * DM], F16, tag="w2T")
                nc.sync.dma_start(w2_sb[:], aps["w2T"])
                q_big = btp.tile([128, NDM * L], F16, tag="tmp1")
                for mc in range(NDM):
                    ps = cps.tile([128, L], F32, tag="psc", name="psc")
                    for fc in range(NFC):
                        for kc in range(NDH):
                            nc.tensor.matmul(
                                ps[:, fc * FC:(fc + 1) * FC],
                                w2_sb[:, kc * DM + mc * 128:
                                      kc * DM + (mc + 1) * 128],
                                yacc[:, kc * L + fc * FC:
                                     kc * L + (fc + 1) * FC],
                                start=(kc == 0), stop=(kc == NDH - 1))
                    nc.scalar.activation(q_big[:, mc * L:(mc + 1) * L],
                                         ps[:], AF.Copy)
                nc.sync.dma_start(
                    qout.rearrange("(c p) l -> p c l", p=128),
                    _cv(q_big[:]))
        m1_cm.__exit__(None, None, None)


_CACHE = {}


def _get_program(rep=1, variant="cc"):
    key = (rep, variant)
    if key not in _CACHE:
        _CACHE[key] = _build_program(rep, variant)
    return _CACHE[key]


def _prep_core_inputs(inp, b, d, half):
    f32 = np.float32
    f16 = np.float16
    pref = "mf" if d == 0 else "mb"
    g = lambda k: np.asarray(inp[f"{pref}_{k}"], f32)
    ln_w = np.asarray(inp["ln_w"], f32)
    ln_b = np.asarray(inp["ln_b"], f32)
    in_w = g("in_w")
    x = np.asarray(inp["x"], f32)[b]
    if d == 1:
        x = x[::-1]
    perm = np.concatenate([np.arange(half * DH, (half + 1) * DH),
                           np.arange((1 - half) * DH, (2 - half) * DH)])
    hs = slice(half * DH, (half + 1) * DH)
    wu = in_w[0:DI][perm]
    wz = in_w[DI + half * DH:DI + (half + 1) * DH]
    A = -np.exp(g("A_log")[hs])
    assert np.abs(A + np.arange(1, NS + 1)).max() < 1e-4, \
        "kernel assumes A[:, n] == -(n+1)"
    xproj = g("xproj_w").T[perm].copy()        # (DI, 80) permuted
    xproj[:, DTR:DTR + NS] *= -1.0             # fold v = -ln(w)*u sign into B

    pb = np.zeros((128, P_END), f32)
    pb[:, P_BU:P_BZ] = (wu @ ln_b).reshape(NDU, 128).T
    pb[:, P_BZ:P_CW] = (wz @ ln_b).reshape(NDH, 128).T
    pb[:, P_CW:P_CB] = (g("conv_w")[perm].reshape(NDU, 128, KC)
                        .transpose(1, 0, 2).reshape(128, -1))
    pb[:, P_CB:P_DTBN] = g("conv_b")[perm].reshape(NDU, 128).T
    pb[:, P_DTBN:P_DV] = -g("dt_b")[hs].reshape(NDH, 128).T
    pb[:, P_DV:P_EPS] = g("D")[hs].reshape(NDH, 128).T
    pb[:, P_EPS:P_END] = 1e-5
    return {
        "xinT": np.ascontiguousarray(x.T.astype(f16)),
        "wuzT": np.ascontiguousarray(
            (np.concatenate([wu.T, wz.T], axis=1)
             * ln_w[:, None]).astype(f16)),
        "pblk": np.ascontiguousarray(pb),
        "xprojT": np.ascontiguousarray(
            xproj.reshape(NDU, 128, 80).transpose(1, 0, 2)
            .reshape(128, -1).astype(f16)),
        "dtwT": np.ascontiguousarray(g("dt_w")[hs].T.astype(f16)),
        "w2T": np.ascontiguousarray(
            (np.asarray(inp["proj_w"], f32)[:, d * DM:(d + 1) * DM]
             @ g("out_w")[:, hs]).T.reshape(NDH, 128, DM).transpose(1, 0, 2)
             .reshape(128, -1).astype(f16)),
    }


_DEFAULT_VARIANT = "cc"


def _run(inp, rep=1, trace=False, variant=None):
    if variant is None:
        variant = _DEFAULT_VARIANT
    nc = _get_program(rep, variant)
    in_maps = []
    for c in range(8):
        b, d, half = c >> 2, (c >> 1) & 1, c & 1
        in_maps.append(_prep_core_inputs(inp, b, d, half))
    return run_bass_kernel_spmd(nc, in_maps, list(range(8)), trace=trace)


def kernel(**inputs):
    global _DEFAULT_VARIANT
    try:
        res = _run(inputs, rep=1)
    except Exception:
        if _DEFAULT_VARIANT == "full":
            raise
        # collectives unavailable in this environment: fall back to the
        # replicated-xproj program (no cross-core communication)
        _DEFAULT_VARIANT = "full"
        res = _run(inputs, rep=1)
    x = np.asarray(inputs["x"], np.float32)
    proj_b = np.asarray(inputs["proj_b"], np.float32)
    out = np.empty((2, L, DM), np.float32)
    for b in range(2):
        acc = x[b] + proj_b
        for d in range(2):
            for half in range(2):
                c = (b << 2) | (d << 1) | half
                q = res.results[c]["q"].astype(np.float32).T   # (L, DM)
                if d == 1:
                    q = q[::-1]
                acc = acc + q
        out[b] = acc
    return out


if __name__ == "__main__":
    nc = _get_program(1)
    print("build ok")
